# revision 7
# baseline (speedup 1.0000x reference)
"""DampedLinOSSLayer Trainium2 kernel (8 NeuronCores, batch-sharded).

Math: per SSM channel p, the complex diagonal recurrence
    x_t = lam_p * x_{t-1} + bu_t,   lam_p = r_p * exp(i*th_p)
is factored through the gauge x_t = exp(i*th_p*t) * y_t:
    y_t = r_p * y_{t-1} + c_t,      c_t = exp(-i*th_p*t) * bu_t
which has a REAL per-channel coefficient -> runs as hardware
tensor_tensor_scan (DVE) on the re/im planes independently.
The phase rotations exp(-/+ i*th*t) are split as t = 512*T + t0:
the chunk part exp(+-i*th*512T) is folded (on host) into per-chunk
copies of the B / C projection weights; only the in-chunk part
exp(+-i*th*t0), t0 in [0,512), is applied on-device as elementwise
multiplies with constant [128, 512] tables.

Layout on device ("ST-form"): SSM channel p on partitions (2 halves of
128), time on the free dim. Per core: 4 batches of the 32.
  - input tiles [l,h] -> PE transpose -> inT [h, l]
  - B-proj:  bu[p_half, t]  = B_J^T.T @ inT        (PE)
  - pre-rotation (packed complex mul)              (DVE)
  - scan y = r*y + c along t, full L=2048          (DVE tensor_tensor_scan)
  - post-rotation -> x                             (DVE)
  - C-proj + D-residual -> out[t, h]               (PE, PSUM-accumulated)
"""

import functools
import numpy as np

BATCH, LENGTH, HIDDEN, P = 32, 2048, 128, 256
N_CORES = 8
BPC = BATCH // N_CORES          # batches per core
CH = 512                        # chunk size (phase fold granularity)
NCH = LENGTH // CH              # 4 chunks
NBLK = CH // 128                # 4 token-blocks of 128 per chunk

_COMPILED = {}


def _build_program(mm_dtype_name="float32"):
    import concourse.bacc as bacc
    import concourse.mybir as mybir
    from concourse.tile import TileContext

    f32 = mybir.dt.float32
    mmdt = getattr(mybir.dt, mm_dtype_name)

    nc = bacc.Bacc("TRN2", target_bir_lowering=False, debug=False,
                   num_devices=N_CORES)

    # ---- DRAM tensors (per-core) ----
    xin = nc.dram_tensor("xin", [BPC, LENGTH, HIDDEN], f32,
                         kind="ExternalInput").ap()
    # B weights, phase-folded per chunk: [J, comp(re/im), half, h, p]
    bw = nc.dram_tensor("bw", [HIDDEN, NCH, 2, 2, 128], f32,
                        kind="ExternalInput").ap()
    # C weights, phase-folded per chunk (sign of im folded): [T, comp, half, p, h]
    cw = nc.dram_tensor("cw", [128, NCH, 2, 2, HIDDEN], f32,
                        kind="ExternalInput").ap()
    # in-chunk rotation tables, packed for the 2-mult complex trick:
    # epre/epost: [half, which(0=T1,1=T2), 128, 2, 512]
    epre = nc.dram_tensor("epre", [128, 2, 2, 2, CH], f32,
                          kind="ExternalInput").ap()
    epost = nc.dram_tensor("epost", [128, 2, 2, 2, CH], f32,
                           kind="ExternalInput").ap()
    rcol = nc.dram_tensor("rcol", [128, 2], f32, kind="ExternalInput").ap()
    dw = nc.dram_tensor("dw", [HIDDEN, HIDDEN], f32, kind="ExternalInput").ap()
    eye = nc.dram_tensor("eye", [128, 128], f32, kind="ExternalInput").ap()
    out = nc.dram_tensor("out", [BPC, LENGTH, HIDDEN], f32,
                         kind="ExternalOutput").ap()

    with TileContext(nc) as tc:
        with (
            tc.tile_pool(name="const", bufs=1) as cpool,
            tc.tile_pool(name="inat", bufs=4) as inat_pool,
            tc.tile_pool(name="intp", bufs=2) as intr_pool,   # inT per batch
            tc.tile_pool(name="cbuf", bufs=2) as cbuf_pool,   # scan in
            tc.tile_pool(name="ybuf", bufs=2) as ybuf_pool,   # scan out
            tc.tile_pool(name="xbuf", bufs=2) as xbuf_pool,   # post-rot
            tc.tile_pool(name="pst", bufs=2, space="PSUM") as pst,
            tc.tile_pool(name="psb", bufs=1, space="PSUM") as psb,
            tc.tile_pool(name="pso", bufs=2, space="PSUM") as pso,
        ):
            # ---- constants to SBUF ----
            bw_t = cpool.tile([HIDDEN, NCH, 2, 2, 128], f32, tag="bw")
            cw_t = cpool.tile([128, NCH, 2, 2, HIDDEN], f32, tag="cw")
            epre_t = cpool.tile([128, 2, 2, 2, CH], f32, tag="epre")
            epost_t = cpool.tile([128, 2, 2, 2, CH], f32, tag="epost")
            rcol_t = cpool.tile([128, 2], f32, tag="rcol")
            dw_t = cpool.tile([HIDDEN, HIDDEN], f32, tag="dw")
            eye_t = cpool.tile([128, 128], f32, tag="eye")
            for src, dst in [(bw, bw_t), (cw, cw_t), (epre, epre_t),
                             (epost, epost_t), (rcol, rcol_t), (dw, dw_t),
                             (eye, eye_t)]:
                nc.sync.dma_start(dst[:], src[:])

            # broadcast r along free dim for the scan coefficient
            rbc = cpool.tile([128, 2, CH], f32, tag="rbc")
            for half in range(2):
                nc.vector.memset(rbc[:, half], 1.0)
                nc.vector.tensor_scalar_mul(
                    rbc[:, half], rbc[:, half], rcol_t[:, half:half + 1])

            for b in range(BPC):
                # ---- load + transpose input: inT [h, l] ----
                inT = intr_pool.tile([HIDDEN, LENGTH], f32, tag="inT")
                for blk in range(LENGTH // 128):
                    nat = inat_pool.tile([128, HIDDEN], f32, tag="nat")
                    nc.sync.dma_start(
                        nat[:], xin[b, 128 * blk:128 * (blk + 1), :])
                    tp = pst.tile([HIDDEN, 128], f32, tag="tp")
                    nc.tensor.transpose(tp[:], nat[:], eye_t[:])
                    nc.scalar.copy(inT[:, 128 * blk:128 * (blk + 1)], tp[:])

                y_prev = [None, None]
                for J in range(NCH):
                    tsl = slice(CH * J, CH * (J + 1))
                    x_sb = []
                    for half in range(2):
                        # ---- B-proj: bu[p_half, 2, CH] (re|im packed) ----
                        bu = psb.tile([128, 2, CH], f32, tag=f"bu{half}")
                        for comp in range(2):
                            nc.tensor.matmul(
                                bu[:, comp, :],
                                bw_t[:, J, comp, half].bitcast(mmdt),
                                inT[:, tsl].bitcast(mmdt),
                                start=True, stop=True)
                        # ---- pre-rotation: c = E- * bu (complex) ----
                        t1 = xbuf_pool.tile([128, 2, CH], f32, tag="t1")
                        t2 = xbuf_pool.tile([128, 2, CH], f32, tag="t2")
                        cc = cbuf_pool.tile([128, 2, CH], f32, tag=f"c{half}",
                                            name=f"c{half}")
                        nc.vector.tensor_mul(t1[:], bu[:], epre_t[:, half, 0])
                        nc.vector.tensor_mul(t2[:], bu[:], epre_t[:, half, 1])
                        nc.vector.tensor_add(
                            cc[:, 0, :], t1[:, 0, :], t1[:, 1, :])
                        nc.vector.tensor_add(
                            cc[:, 1, :], t2[:, 0, :], t2[:, 1, :])

                        # ---- chained scan: y = r * y_prev + c ----
                        yy = ybuf_pool.tile([128, 2, CH], f32, tag=f"y{half}",
                                            name=f"y{half}")
                        for comp in range(2):
                            init = (0.0 if y_prev[half] is None else
                                    y_prev[half][:, comp, CH - 1:CH])
                            nc.vector.tensor_tensor_scan(
                                yy[:, comp, :],
                                rbc[:, half],
                                cc[:, comp, :],
                                init,
                                op0=mybir.AluOpType.mult,
                                op1=mybir.AluOpType.add)
                        y_prev[half] = yy

                        # ---- post-rotation: x = E+ * y (complex) ----
                        xs = xbuf_pool.tile([128, 2, CH], f32, tag=f"x{half}",
                                            name=f"x{half}")
                        t3 = xbuf_pool.tile([128, 2, CH], f32, tag="t3")
                        t4 = xbuf_pool.tile([128, 2, CH], f32, tag="t4")
                        nc.vector.tensor_mul(t3[:], yy[:], epost_t[:, half, 0])
                        nc.vector.tensor_mul(t4[:], yy[:], epost_t[:, half, 1])
                        nc.vector.tensor_add(
                            xs[:, 0, :], t3[:, 0, :], t3[:, 1, :])
                        nc.vector.tensor_add(
                            xs[:, 1, :], t4[:, 0, :], t4[:, 1, :])
                        x_sb.append(xs)

                    # ---- C-proj + D-residual per 128-token block ----
                    for i in range(NBLK):
                        bsl = slice(128 * i, 128 * (i + 1))
                        gsl = slice(CH * J + 128 * i, CH * J + 128 * (i + 1))
                        ops = pso.tile([128, HIDDEN], f32, tag="ops")
                        first = True
                        for comp in range(2):
                            for half in range(2):
                                nc.tensor.matmul(
                                    ops[:],
                                    x_sb[half][:, comp, bsl].bitcast(mmdt),
                                    cw_t[:, J, comp, half].bitcast(mmdt),
                                    start=first, stop=False)
                                first = False
                        nc.tensor.matmul(
                            ops[:], inT[:, gsl].bitcast(mmdt),
                            dw_t[:].bitcast(mmdt), start=False, stop=True)
                        osb = xbuf_pool.tile([128, HIDDEN], f32, tag="osb",
                                             bufs=4)
                        nc.scalar.copy(osb[:], ops[:])
                        nc.sync.dma_start(out[b, gsl, :], osb[:])

    nc.compile()
    return nc


def _host_constants(A_diag, G_diag, steps, B, C, D):
    """Parameter projection + eigenvalues + phase-folded weight tables."""
    A = A_diag.astype(np.float64)
    G = G_diag.astype(np.float64)
    st = steps.astype(np.float64)
    step = 1.0 / (1.0 + np.exp(-st))
    g = np.maximum(G, 0.0)
    denom = np.maximum(step * step, 1e-6)
    s = step * g
    base = np.sqrt(np.maximum(1.0 + s, 1e-6))
    a_low = (2.0 + s - 2.0 * base) / denom
    a_high = (2.0 + s + 2.0 * base) / denom
    a = a_low + np.maximum(A - a_low, 0.0) - np.maximum(A - a_high, 0.0)
    S = 1.0 / (1.0 + step * g)
    T = S + 1.0 - step * step * S * a
    imag = np.sqrt(np.maximum(S - 0.25 * T * T, 0.0))
    lam = 0.5 * T + 1j * imag                      # [P] complex128
    r = np.abs(lam)
    th = np.angle(lam)

    j0 = np.arange(CH, dtype=np.float64)
    # in-chunk rotations, [P, CH]
    cos_m = np.cos(th[:, None] * j0[None, :])
    sin_m = np.sin(th[:, None] * j0[None, :])

    # epre packs for c = exp(-i th t0') * bu:
    #   cre = bur*cos + bui*sin   -> T1 mult table [cos | sin], add halves
    #   cim = bur*(-sin) + bui*cos-> T2 mult table [-sin | cos], add halves
    # epost for x = exp(+i th t0') * y:
    #   xr = yr*cos + yi*(-sin)   -> T1 table [cos | -sin]
    #   xi = yr*sin + yi*cos      -> T2 table [sin | cos]
    epre = np.zeros((128, 2, 2, 2, CH), np.float32)
    epost = np.zeros((128, 2, 2, 2, CH), np.float32)
    for half in range(2):
        psl = slice(128 * half, 128 * (half + 1))
        epre[:, half, 0, 0, :] = cos_m[psl]
        epre[:, half, 0, 1, :] = sin_m[psl]
        epre[:, half, 1, 0, :] = -sin_m[psl]
        epre[:, half, 1, 1, :] = cos_m[psl]
        epost[:, half, 0, 0, :] = cos_m[psl]
        epost[:, half, 0, 1, :] = -sin_m[psl]
        epost[:, half, 1, 0, :] = sin_m[psl]
        epost[:, half, 1, 1, :] = cos_m[psl]

    # chunk-folded B: B_J = exp(-i th * CH * J) * (Br + i Bi)
    Bc = B[..., 0].astype(np.float64) + 1j * B[..., 1].astype(np.float64)
    Cc = C[..., 0].astype(np.float64) + 1j * C[..., 1].astype(np.float64)
    bw = np.zeros((HIDDEN, NCH, 2, 2, 128), np.float32)
    cwt = np.zeros((128, NCH, 2, 2, HIDDEN), np.float32)
    for J in range(NCH):
        ph = np.exp(-1j * th * (CH * J))           # [P]
        BJ = Bc * ph[:, None]                      # [P, H]
        phc = np.exp(+1j * th * (CH * J))
        CT = Cc * phc[None, :]                     # [H, P] (C' = Cre + i Cim)
        for half in range(2):
            psl = slice(128 * half, 128 * (half + 1))
            bw[:, J, 0, half] = BJ.real[psl].T     # lhsT [h, p]
            bw[:, J, 1, half] = BJ.imag[psl].T
            # out = Re{C'_T x} = CTre*xr - CTim*xi ; rhs [p, h]
            cwt[:, J, 0, half] = CT.real[:, psl].T
            cwt[:, J, 1, half] = -CT.imag[:, psl].T

    rcol = np.zeros((128, 2), np.float32)
    rcol[:, 0] = r[:128]
    rcol[:, 1] = r[128:]
    dwm = np.diag(D.astype(np.float64)).astype(np.float32)
    eye = np.eye(128, dtype=np.float32)
    return dict(bw=bw, cw=cwt, epre=epre, epost=epost, rcol=rcol, dw=dwm,
                eye=eye)


def kernel(inputs, A_diag, G_diag, steps, B, C, D):
    from concourse import bass_utils

    inputs = np.asarray(inputs, np.float32)
    consts = _host_constants(np.asarray(A_diag), np.asarray(G_diag),
                             np.asarray(steps), np.asarray(B), np.asarray(C),
                             np.asarray(D))

    if "prog" not in _COMPILED:
        _COMPILED["prog"] = _build_program()
    nc = _COMPILED["prog"]

    in_maps = []
    for core in range(N_CORES):
        m = dict(consts)
        m["xin"] = np.ascontiguousarray(inputs[BPC * core: BPC * (core + 1)])
        in_maps.append(m)
    res = bass_utils.run_bass_kernel_spmd(nc, in_maps,
                                          core_ids=list(range(N_CORES)))
    out = np.concatenate([res.results[i]["out"] for i in range(N_CORES)],
                         axis=0)
    return out.astype(np.float32)


# revision 9
# speedup vs baseline: 2629.3873x; 2629.3873x over previous
"""DampedLinOSSLayer Trainium2 kernel (8 NeuronCores, batch-sharded).

Math: per SSM channel p, the complex diagonal recurrence
    x_t = lam_p * x_{t-1} + bu_t,   lam_p = r_p * exp(i*th_p)
is factored through the gauge x_t = exp(i*th_p*t) * y_t:
    y_t = r_p * y_{t-1} + c_t,      c_t = exp(-i*th_p*t) * bu_t
which has a REAL per-channel coefficient -> runs as hardware
tensor_tensor_scan (DVE) on the re/im planes independently.
The phase rotations exp(-/+ i*th*t) are split as t = 512*T + t0:
the chunk part exp(+-i*th*512T) is folded (on host) into per-chunk
copies of the B / C projection weights; only the in-chunk part
exp(+-i*th*t0), t0 in [0,512), is applied on-device as elementwise
multiplies with constant [128, 512] tables.

Layout on device ("ST-form"): SSM channel p on partitions (2 halves of
128), time on the free dim. Per core: 4 batches of the 32.
  - input tiles [l,h] -> PE transpose -> inT [h, l]
  - B-proj:  bu[p_half, t]  = B_J^T.T @ inT        (PE)
  - pre-rotation (packed complex mul)              (DVE)
  - scan y = r*y + c along t, full L=2048          (DVE tensor_tensor_scan)
  - post-rotation -> x                             (DVE)
  - C-proj + D-residual -> out[t, h]               (PE, PSUM-accumulated)
"""

import functools
import numpy as np

BATCH, LENGTH, HIDDEN, P = 32, 2048, 128, 256
N_CORES = 8
BPC = BATCH // N_CORES          # batches per core
CH = 512                        # chunk size (phase fold granularity)
NCH = LENGTH // CH              # 4 chunks
NBLK = CH // 128                # 4 token-blocks of 128 per chunk

_COMPILED = {}


def _build_program(mm_dtype_name="float32", reps=1, skip=()):
    """reps>1 wraps the whole per-core body in a hardware loop (timing).
    skip: subset of {"dve", "pe_bc", "tr"} disabling sections (timing)."""
    import concourse.bacc as bacc
    import concourse.mybir as mybir
    from concourse.tile import TileContext

    f32 = mybir.dt.float32
    mmdt = getattr(mybir.dt, mm_dtype_name)

    nc = bacc.Bacc("TRN2", target_bir_lowering=False, debug=False,
                   num_devices=N_CORES)

    # ---- DRAM tensors (per-core) ----
    xin = nc.dram_tensor("xin", [BPC, LENGTH, HIDDEN], f32,
                         kind="ExternalInput").ap()
    # B weights, phase-folded per chunk: [J, comp(re/im), half, h, p]
    bw = nc.dram_tensor("bw", [HIDDEN, NCH, 2, 2, 128], f32,
                        kind="ExternalInput").ap()
    # C weights, phase-folded per chunk (sign of im folded): [T, comp, half, p, h]
    cw = nc.dram_tensor("cw", [128, NCH, 2, 2, HIDDEN], f32,
                        kind="ExternalInput").ap()
    # in-chunk rotation tables, packed for the 2-mult complex trick:
    # epre/epost: [half, which(0=T1,1=T2), 128, 2, 512]
    epre = nc.dram_tensor("epre", [128, 2, 2, 2, CH], f32,
                          kind="ExternalInput").ap()
    epost = nc.dram_tensor("epost", [128, 2, 2, 2, CH], f32,
                           kind="ExternalInput").ap()
    rcol = nc.dram_tensor("rcol", [128, 2], f32, kind="ExternalInput").ap()
    dw = nc.dram_tensor("dw", [HIDDEN, HIDDEN], f32, kind="ExternalInput").ap()
    eye = nc.dram_tensor("eye", [128, 128], f32, kind="ExternalInput").ap()
    out = nc.dram_tensor("out", [BPC, LENGTH, HIDDEN], f32,
                         kind="ExternalOutput").ap()

    with TileContext(nc) as tc:
        import contextlib

        @contextlib.contextmanager
        def body_loop():
            if reps == 1:
                yield
            else:
                with tc.For_i(0, reps, 1):
                    yield

        with (
            tc.tile_pool(name="const", bufs=1) as cpool,
            tc.tile_pool(name="inat", bufs=4) as inat_pool,
            tc.tile_pool(name="intp", bufs=2) as intr_pool,   # inT per batch
            tc.tile_pool(name="cbuf", bufs=2) as cbuf_pool,   # scan in
            tc.tile_pool(name="ybuf", bufs=2) as ybuf_pool,   # scan out
            tc.tile_pool(name="xbuf", bufs=2) as xbuf_pool,   # post-rot
            tc.tile_pool(name="pst", bufs=2, space="PSUM") as pst,
            tc.tile_pool(name="psb", bufs=1, space="PSUM") as psb,
            tc.tile_pool(name="pso", bufs=2, space="PSUM") as pso,
        ):
            # ---- constants to SBUF ----
            bw_t = cpool.tile([HIDDEN, NCH, 2, 2, 128], f32, tag="bw")
            cw_t = cpool.tile([128, NCH, 2, 2, HIDDEN], f32, tag="cw")
            epre_t = cpool.tile([128, 2, 2, 2, CH], f32, tag="epre")
            epost_t = cpool.tile([128, 2, 2, 2, CH], f32, tag="epost")
            rcol_t = cpool.tile([128, 2], f32, tag="rcol")
            dw_t = cpool.tile([HIDDEN, HIDDEN], f32, tag="dw")
            eye_t = cpool.tile([128, 128], f32, tag="eye")
            for src, dst in [(bw, bw_t), (cw, cw_t), (epre, epre_t),
                             (epost, epost_t), (rcol, rcol_t), (dw, dw_t),
                             (eye, eye_t)]:
                nc.sync.dma_start(dst[:], src[:])

            # broadcast r along free dim for the scan coefficient
            rbc = cpool.tile([128, 2, CH], f32, tag="rbc")
            for half in range(2):
                nc.vector.memset(rbc[:, half], 1.0)
                nc.vector.tensor_scalar_mul(
                    rbc[:, half], rbc[:, half], rcol_t[:, half:half + 1])

            ctx_loop = body_loop()
            ctx_loop.__enter__()
            for b in range(BPC):
                # ---- load + transpose input: inT [h, l] ----
                inT = intr_pool.tile([HIDDEN, LENGTH], f32, tag="inT")
                for blk in range(LENGTH // 128):
                    nat = inat_pool.tile([128, HIDDEN], f32, tag="nat")
                    nc.sync.dma_start(
                        nat[:], xin[b, 128 * blk:128 * (blk + 1), :])
                    if "tr" not in skip:
                        tp = pst.tile([HIDDEN, 128], f32, tag="tp")
                        nc.tensor.transpose(tp[:], nat[:], eye_t[:])
                        nc.scalar.copy(
                            inT[:, 128 * blk:128 * (blk + 1)], tp[:])

                y_prev = [None, None]
                for J in range(NCH):
                    tsl = slice(CH * J, CH * (J + 1))
                    x_sb = []
                    for half in range(2):
                        # ---- B-proj: bu[p_half, 2, CH] (re|im packed) ----
                        bu = psb.tile([128, 2, CH], f32, tag=f"bu{half}")
                        if "pe_bc" not in skip:
                            for comp in range(2):
                                nc.tensor.matmul(
                                    bu[:, comp, :],
                                    bw_t[:, J, comp, half].bitcast(mmdt),
                                    inT[:, tsl].bitcast(mmdt),
                                    start=True, stop=True)
                        # ---- pre-rotation: c = E- * bu (complex) ----
                        t1 = xbuf_pool.tile([128, 2, CH], f32, tag="t1")
                        t2 = xbuf_pool.tile([128, 2, CH], f32, tag="t2")
                        cc = cbuf_pool.tile([128, 2, CH], f32, tag=f"c{half}",
                                            name=f"c{half}")
                        if "dve" not in skip:
                            nc.vector.tensor_mul(
                                t1[:], bu[:], epre_t[:, half, 0])
                            nc.vector.tensor_mul(
                                t2[:], bu[:], epre_t[:, half, 1])
                            nc.vector.tensor_add(
                                cc[:, 0, :], t1[:, 0, :], t1[:, 1, :])
                            nc.vector.tensor_add(
                                cc[:, 1, :], t2[:, 0, :], t2[:, 1, :])

                        # ---- chained scan: y = r * y_prev + c ----
                        yy = ybuf_pool.tile([128, 2, CH], f32, tag=f"y{half}",
                                            name=f"y{half}")
                        if "dve" not in skip:
                            for comp in range(2):
                                init = (0.0 if y_prev[half] is None else
                                        y_prev[half][:, comp, CH - 1:CH])
                                nc.vector.tensor_tensor_scan(
                                    yy[:, comp, :],
                                    rbc[:, half],
                                    cc[:, comp, :],
                                    init,
                                    op0=mybir.AluOpType.mult,
                                    op1=mybir.AluOpType.add)
                        y_prev[half] = yy

                        # ---- post-rotation: x = E+ * y (complex) ----
                        xs = xbuf_pool.tile([128, 2, CH], f32, tag=f"x{half}",
                                            name=f"x{half}")
                        t3 = xbuf_pool.tile([128, 2, CH], f32, tag="t3")
                        t4 = xbuf_pool.tile([128, 2, CH], f32, tag="t4")
                        if "dve" not in skip:
                            nc.vector.tensor_mul(
                                t3[:], yy[:], epost_t[:, half, 0])
                            nc.vector.tensor_mul(
                                t4[:], yy[:], epost_t[:, half, 1])
                            nc.vector.tensor_add(
                                xs[:, 0, :], t3[:, 0, :], t3[:, 1, :])
                            nc.vector.tensor_add(
                                xs[:, 1, :], t4[:, 0, :], t4[:, 1, :])
                        x_sb.append(xs)

                    # ---- C-proj + D-residual per 128-token block ----
                    for i in range(NBLK):
                        bsl = slice(128 * i, 128 * (i + 1))
                        gsl = slice(CH * J + 128 * i, CH * J + 128 * (i + 1))
                        ops = pso.tile([128, HIDDEN], f32, tag="ops")
                        if "pe_bc" not in skip:
                            first = True
                            for comp in range(2):
                                for half in range(2):
                                    nc.tensor.matmul(
                                        ops[:],
                                        x_sb[half][:, comp, bsl].bitcast(mmdt),
                                        cw_t[:, J, comp, half].bitcast(mmdt),
                                        start=first, stop=False)
                                    first = False
                            nc.tensor.matmul(
                                ops[:], inT[:, gsl].bitcast(mmdt),
                                dw_t[:].bitcast(mmdt), start=False, stop=True)
                        osb = xbuf_pool.tile([128, HIDDEN], f32, tag="osb",
                                             bufs=4)
                        nc.scalar.copy(osb[:], ops[:])
                        nc.sync.dma_start(out[b, gsl, :], osb[:])

            ctx_loop.__exit__(None, None, None)

    nc.compile()
    return nc


def _host_constants(A_diag, G_diag, steps, B, C, D):
    """Parameter projection + eigenvalues + phase-folded weight tables."""
    A = A_diag.astype(np.float64)
    G = G_diag.astype(np.float64)
    st = steps.astype(np.float64)
    step = 1.0 / (1.0 + np.exp(-st))
    g = np.maximum(G, 0.0)
    denom = np.maximum(step * step, 1e-6)
    s = step * g
    base = np.sqrt(np.maximum(1.0 + s, 1e-6))
    a_low = (2.0 + s - 2.0 * base) / denom
    a_high = (2.0 + s + 2.0 * base) / denom
    a = a_low + np.maximum(A - a_low, 0.0) - np.maximum(A - a_high, 0.0)
    S = 1.0 / (1.0 + step * g)
    T = S + 1.0 - step * step * S * a
    imag = np.sqrt(np.maximum(S - 0.25 * T * T, 0.0))
    lam = 0.5 * T + 1j * imag                      # [P] complex128
    r = np.abs(lam)
    th = np.angle(lam)

    j0 = np.arange(CH, dtype=np.float64)
    # in-chunk rotations, [P, CH]
    cos_m = np.cos(th[:, None] * j0[None, :])
    sin_m = np.sin(th[:, None] * j0[None, :])

    # epre packs for c = exp(-i th t0') * bu:
    #   cre = bur*cos + bui*sin   -> T1 mult table [cos | sin], add halves
    #   cim = bur*(-sin) + bui*cos-> T2 mult table [-sin | cos], add halves
    # epost for x = exp(+i th t0') * y:
    #   xr = yr*cos + yi*(-sin)   -> T1 table [cos | -sin]
    #   xi = yr*sin + yi*cos      -> T2 table [sin | cos]
    epre = np.zeros((128, 2, 2, 2, CH), np.float32)
    epost = np.zeros((128, 2, 2, 2, CH), np.float32)
    for half in range(2):
        psl = slice(128 * half, 128 * (half + 1))
        epre[:, half, 0, 0, :] = cos_m[psl]
        epre[:, half, 0, 1, :] = sin_m[psl]
        epre[:, half, 1, 0, :] = -sin_m[psl]
        epre[:, half, 1, 1, :] = cos_m[psl]
        epost[:, half, 0, 0, :] = cos_m[psl]
        epost[:, half, 0, 1, :] = -sin_m[psl]
        epost[:, half, 1, 0, :] = sin_m[psl]
        epost[:, half, 1, 1, :] = cos_m[psl]

    # chunk-folded B: B_J = exp(-i th * CH * J) * (Br + i Bi)
    Bc = B[..., 0].astype(np.float64) + 1j * B[..., 1].astype(np.float64)
    Cc = C[..., 0].astype(np.float64) + 1j * C[..., 1].astype(np.float64)
    bw = np.zeros((HIDDEN, NCH, 2, 2, 128), np.float32)
    cwt = np.zeros((128, NCH, 2, 2, HIDDEN), np.float32)
    for J in range(NCH):
        ph = np.exp(-1j * th * (CH * J))           # [P]
        BJ = Bc * ph[:, None]                      # [P, H]
        phc = np.exp(+1j * th * (CH * J))
        CT = Cc * phc[None, :]                     # [H, P] (C' = Cre + i Cim)
        for half in range(2):
            psl = slice(128 * half, 128 * (half + 1))
            bw[:, J, 0, half] = BJ.real[psl].T     # lhsT [h, p]
            bw[:, J, 1, half] = BJ.imag[psl].T
            # out = Re{C'_T x} = CTre*xr - CTim*xi ; rhs [p, h]
            cwt[:, J, 0, half] = CT.real[:, psl].T
            cwt[:, J, 1, half] = -CT.imag[:, psl].T

    rcol = np.zeros((128, 2), np.float32)
    rcol[:, 0] = r[:128]
    rcol[:, 1] = r[128:]
    dwm = np.diag(D.astype(np.float64)).astype(np.float32)
    eye = np.eye(128, dtype=np.float32)
    return dict(bw=bw, cw=cwt, epre=epre, epost=epost, rcol=rcol, dw=dwm,
                eye=eye)


def kernel(inputs, A_diag, G_diag, steps, B, C, D):
    from concourse import bass_utils

    inputs = np.asarray(inputs, np.float32)
    consts = _host_constants(np.asarray(A_diag), np.asarray(G_diag),
                             np.asarray(steps), np.asarray(B), np.asarray(C),
                             np.asarray(D))

    if "prog" not in _COMPILED:
        _COMPILED["prog"] = _build_program()
    nc = _COMPILED["prog"]

    in_maps = []
    for core in range(N_CORES):
        m = dict(consts)
        m["xin"] = np.ascontiguousarray(inputs[BPC * core: BPC * (core + 1)])
        in_maps.append(m)
    res = bass_utils.run_bass_kernel_spmd(nc, in_maps,
                                          core_ids=list(range(N_CORES)))
    out = np.concatenate([res.results[i]["out"] for i in range(N_CORES)],
                         axis=0)
    return out.astype(np.float32)


# revision 10
# speedup vs baseline: 8560.3267x; 3.2556x over previous
"""DampedLinOSSLayer Trainium2 kernel (8 NeuronCores, batch-sharded).

Math: per SSM channel p, the complex diagonal recurrence
    x_t = lam_p * x_{t-1} + bu_t,   lam_p = r_p * exp(i*th_p)
is factored through the gauge x_t = exp(i*th_p*t) * y_t:
    y_t = r_p * y_{t-1} + c_t,      c_t = exp(-i*th_p*t) * bu_t
which has a REAL per-channel coefficient -> runs as hardware
tensor_tensor_scan (DVE) on the re/im planes independently.
The phase rotations exp(-/+ i*th*t) are split as t = 512*T + t0:
the chunk part exp(+-i*th*512T) is folded (on host) into per-chunk
copies of the B / C projection weights; only the in-chunk part
exp(+-i*th*t0), t0 in [0,512), is applied on-device as elementwise
multiplies with constant [128, 512] tables.

Layout on device ("ST-form"): SSM channel p on partitions (2 halves of
128), time on the free dim. Per core: 4 batches of the 32.
  - input tiles [l,h] -> PE transpose -> inT [h, l]
  - B-proj:  bu[p_half, t]  = B_J^T.T @ inT        (PE)
  - pre-rotation (packed complex mul)              (DVE)
  - scan y = r*y + c along t, full L=2048          (DVE tensor_tensor_scan)
  - post-rotation -> x                             (DVE)
  - C-proj + D-residual -> out[t, h]               (PE, PSUM-accumulated)
"""

import functools
import numpy as np

BATCH, LENGTH, HIDDEN, P = 32, 2048, 128, 256
N_CORES = 8
BPC = BATCH // N_CORES          # batches per core
CH = 512                        # chunk size (phase fold granularity)
NCH = LENGTH // CH              # 4 chunks
NBLK = CH // 128                # 4 token-blocks of 128 per chunk

_COMPILED = {}


def _build_program(mm_dtype_name="float32", reps=1, skip=()):
    """reps>1 wraps the whole per-core body in a hardware loop (timing).
    skip: subset of {"dve", "pe_bc", "tr"} disabling sections (timing)."""
    import concourse.bacc as bacc
    import concourse.mybir as mybir
    from concourse.tile import TileContext

    f32 = mybir.dt.float32
    mmdt = getattr(mybir.dt, mm_dtype_name)

    nc = bacc.Bacc("TRN2", target_bir_lowering=False, debug=False,
                   num_devices=N_CORES)

    # ---- DRAM tensors (per-core) ----
    xin = nc.dram_tensor("xin", [BPC, LENGTH, HIDDEN], f32,
                         kind="ExternalInput").ap()
    # B weights, phase-folded per chunk: [J, comp(re/im), half, h, p]
    bw = nc.dram_tensor("bw", [HIDDEN, NCH, 2, 2, 128], f32,
                        kind="ExternalInput").ap()
    # C weights, phase-folded per chunk (sign of im folded): [T, comp, half, p, h]
    cw = nc.dram_tensor("cw", [128, NCH, 2, 2, HIDDEN], f32,
                        kind="ExternalInput").ap()
    # in-chunk rotation tables, packed for the 2-mult complex trick:
    # epre/epost: [half, which(0=T1,1=T2), 128, 2, 512]
    epre = nc.dram_tensor("epre", [128, 2, 2, 2, CH], f32,
                          kind="ExternalInput").ap()
    epost = nc.dram_tensor("epost", [128, 2, 2, 2, CH], f32,
                           kind="ExternalInput").ap()
    rcol = nc.dram_tensor("rcol", [128, 2], f32, kind="ExternalInput").ap()
    dw = nc.dram_tensor("dw", [HIDDEN, HIDDEN], f32, kind="ExternalInput").ap()
    eye = nc.dram_tensor("eye", [128, 128], f32, kind="ExternalInput").ap()
    out = nc.dram_tensor("out", [BPC, LENGTH, HIDDEN], f32,
                         kind="ExternalOutput").ap()

    with TileContext(nc) as tc:
        import contextlib

        @contextlib.contextmanager
        def body_loop():
            if reps == 1:
                yield
            else:
                with tc.For_i(0, reps, 1):
                    yield

        with (
            tc.tile_pool(name="const", bufs=1) as cpool,
            tc.tile_pool(name="inat", bufs=4) as inat_pool,
            tc.tile_pool(name="intp", bufs=2) as intr_pool,   # inT per batch
            tc.tile_pool(name="cbuf", bufs=2) as cbuf_pool,   # scan in
            tc.tile_pool(name="ybuf", bufs=2) as ybuf_pool,   # scan out
            tc.tile_pool(name="xbuf", bufs=2) as xbuf_pool,   # post-rot
            tc.tile_pool(name="pst", bufs=2, space="PSUM") as pst,
            tc.tile_pool(name="psb", bufs=1, space="PSUM") as psb,
            tc.tile_pool(name="pso", bufs=2, space="PSUM") as pso,
        ):
            # ---- constants to SBUF ----
            bw_t = cpool.tile([HIDDEN, NCH, 2, 2, 128], f32, tag="bw")
            cw_t = cpool.tile([128, NCH, 2, 2, HIDDEN], f32, tag="cw")
            epre_t = cpool.tile([128, 2, 2, 2, CH], f32, tag="epre")
            epost_t = cpool.tile([128, 2, 2, 2, CH], f32, tag="epost")
            rcol_t = cpool.tile([128, 2], f32, tag="rcol")
            dw_t = cpool.tile([HIDDEN, HIDDEN], f32, tag="dw")
            eye_t = cpool.tile([128, 128], f32, tag="eye")
            for src, dst in [(bw, bw_t), (cw, cw_t), (epre, epre_t),
                             (epost, epost_t), (rcol, rcol_t), (dw, dw_t),
                             (eye, eye_t)]:
                nc.sync.dma_start(dst[:], src[:])

            # broadcast r along free dim for the scan coefficient
            rbc = cpool.tile([128, 2, CH], f32, tag="rbc")
            for half in range(2):
                nc.vector.memset(rbc[:, half], 1.0)
                nc.vector.tensor_scalar_mul(
                    rbc[:, half], rbc[:, half], rcol_t[:, half:half + 1])

            ctx_loop = body_loop()
            ctx_loop.__enter__()
            for b in range(BPC):
                # ---- load + transpose input: inT [h, l] ----
                inT = intr_pool.tile([HIDDEN, LENGTH], f32, tag="inT")
                for blk in range(LENGTH // 128):
                    nat = inat_pool.tile([128, HIDDEN], f32, tag="nat")
                    nc.sync.dma_start(
                        nat[:], xin[b, 128 * blk:128 * (blk + 1), :])
                    if "tr" not in skip:
                        tp = pst.tile([HIDDEN, 128], f32, tag="tp")
                        nc.tensor.transpose(tp[:], nat[:], eye_t[:])
                        nc.scalar.copy(
                            inT[:, 128 * blk:128 * (blk + 1)], tp[:])

                y_prev = [None, None]
                for J in range(NCH):
                    tsl = slice(CH * J, CH * (J + 1))
                    x_sb = []
                    for half in range(2):
                        # ---- B-proj: bu[p_half, 2, CH] (re|im packed) ----
                        bu = psb.tile([128, 2, CH], f32, tag=f"bu{half}")
                        if "pe_bc" not in skip:
                            for comp in range(2):
                                nc.tensor.matmul(
                                    bu[:, comp, :],
                                    bw_t[:, J, comp, half].bitcast(mmdt),
                                    inT[:, tsl].bitcast(mmdt),
                                    start=True, stop=True)
                        # ---- pre-rotation: c = E- * bu (complex) ----
                        if "dve" not in skip:
                            t1 = xbuf_pool.tile([128, 2, CH], f32, tag="t1")
                            t2 = xbuf_pool.tile([128, 2, CH], f32, tag="t2")
                            cc = cbuf_pool.tile(
                                [128, 2, CH], f32, tag=f"c{half}",
                                name=f"c{half}")
                            nc.vector.tensor_mul(
                                t1[:], bu[:], epre_t[:, half, 0])
                            nc.vector.tensor_mul(
                                t2[:], bu[:], epre_t[:, half, 1])
                            nc.vector.tensor_add(
                                cc[:, 0, :], t1[:, 0, :], t1[:, 1, :])
                            nc.vector.tensor_add(
                                cc[:, 1, :], t2[:, 0, :], t2[:, 1, :])

                        # ---- chained scan: y = r * y_prev + c ----
                        if "dve" not in skip:
                            yy = ybuf_pool.tile(
                                [128, 2, CH], f32, tag=f"y{half}",
                                name=f"y{half}")
                            for comp in range(2):
                                init = (0.0 if y_prev[half] is None else
                                        y_prev[half][:, comp, CH - 1:CH])
                                nc.vector.tensor_tensor_scan(
                                    yy[:, comp, :],
                                    rbc[:, half],
                                    cc[:, comp, :],
                                    init,
                                    op0=mybir.AluOpType.mult,
                                    op1=mybir.AluOpType.add)
                            y_prev[half] = yy

                        # ---- post-rotation: x = E+ * y (complex) ----
                        xs = xbuf_pool.tile([128, 2, CH], f32, tag=f"x{half}",
                                            name=f"x{half}")
                        if "dve" in skip:
                            nc.gpsimd.memset(xs[:], 0.0)
                        else:
                            t3 = xbuf_pool.tile([128, 2, CH], f32, tag="t3")
                            t4 = xbuf_pool.tile([128, 2, CH], f32, tag="t4")
                            nc.vector.tensor_mul(
                                t3[:], yy[:], epost_t[:, half, 0])
                            nc.vector.tensor_mul(
                                t4[:], yy[:], epost_t[:, half, 1])
                            nc.vector.tensor_add(
                                xs[:, 0, :], t3[:, 0, :], t3[:, 1, :])
                            nc.vector.tensor_add(
                                xs[:, 1, :], t4[:, 0, :], t4[:, 1, :])
                        x_sb.append(xs)

                    # ---- C-proj + D-residual per 128-token block ----
                    for i in range(NBLK):
                        bsl = slice(128 * i, 128 * (i + 1))
                        gsl = slice(CH * J + 128 * i, CH * J + 128 * (i + 1))
                        ops = pso.tile([128, HIDDEN], f32, tag="ops")
                        if "pe_bc" not in skip:
                            first = True
                            for comp in range(2):
                                for half in range(2):
                                    nc.tensor.matmul(
                                        ops[:],
                                        x_sb[half][:, comp, bsl].bitcast(mmdt),
                                        cw_t[:, J, comp, half].bitcast(mmdt),
                                        start=first, stop=False)
                                    first = False
                            nc.tensor.matmul(
                                ops[:], inT[:, gsl].bitcast(mmdt),
                                dw_t[:].bitcast(mmdt), start=False, stop=True)
                        osb = xbuf_pool.tile([128, HIDDEN], f32, tag="osb",
                                             bufs=4)
                        nc.scalar.copy(osb[:], ops[:])
                        nc.sync.dma_start(out[b, gsl, :], osb[:])

            ctx_loop.__exit__(None, None, None)

    nc.compile()
    return nc


def _host_constants(A_diag, G_diag, steps, B, C, D):
    """Parameter projection + eigenvalues + phase-folded weight tables."""
    A = A_diag.astype(np.float64)
    G = G_diag.astype(np.float64)
    st = steps.astype(np.float64)
    step = 1.0 / (1.0 + np.exp(-st))
    g = np.maximum(G, 0.0)
    denom = np.maximum(step * step, 1e-6)
    s = step * g
    base = np.sqrt(np.maximum(1.0 + s, 1e-6))
    a_low = (2.0 + s - 2.0 * base) / denom
    a_high = (2.0 + s + 2.0 * base) / denom
    a = a_low + np.maximum(A - a_low, 0.0) - np.maximum(A - a_high, 0.0)
    S = 1.0 / (1.0 + step * g)
    T = S + 1.0 - step * step * S * a
    imag = np.sqrt(np.maximum(S - 0.25 * T * T, 0.0))
    lam = 0.5 * T + 1j * imag                      # [P] complex128
    r = np.abs(lam)
    th = np.angle(lam)

    j0 = np.arange(CH, dtype=np.float64)
    # in-chunk rotations, [P, CH]
    cos_m = np.cos(th[:, None] * j0[None, :])
    sin_m = np.sin(th[:, None] * j0[None, :])

    # epre packs for c = exp(-i th t0') * bu:
    #   cre = bur*cos + bui*sin   -> T1 mult table [cos | sin], add halves
    #   cim = bur*(-sin) + bui*cos-> T2 mult table [-sin | cos], add halves
    # epost for x = exp(+i th t0') * y:
    #   xr = yr*cos + yi*(-sin)   -> T1 table [cos | -sin]
    #   xi = yr*sin + yi*cos      -> T2 table [sin | cos]
    epre = np.zeros((128, 2, 2, 2, CH), np.float32)
    epost = np.zeros((128, 2, 2, 2, CH), np.float32)
    for half in range(2):
        psl = slice(128 * half, 128 * (half + 1))
        epre[:, half, 0, 0, :] = cos_m[psl]
        epre[:, half, 0, 1, :] = sin_m[psl]
        epre[:, half, 1, 0, :] = -sin_m[psl]
        epre[:, half, 1, 1, :] = cos_m[psl]
        epost[:, half, 0, 0, :] = cos_m[psl]
        epost[:, half, 0, 1, :] = -sin_m[psl]
        epost[:, half, 1, 0, :] = sin_m[psl]
        epost[:, half, 1, 1, :] = cos_m[psl]

    # chunk-folded B: B_J = exp(-i th * CH * J) * (Br + i Bi)
    Bc = B[..., 0].astype(np.float64) + 1j * B[..., 1].astype(np.float64)
    Cc = C[..., 0].astype(np.float64) + 1j * C[..., 1].astype(np.float64)
    bw = np.zeros((HIDDEN, NCH, 2, 2, 128), np.float32)
    cwt = np.zeros((128, NCH, 2, 2, HIDDEN), np.float32)
    for J in range(NCH):
        ph = np.exp(-1j * th * (CH * J))           # [P]
        BJ = Bc * ph[:, None]                      # [P, H]
        phc = np.exp(+1j * th * (CH * J))
        CT = Cc * phc[None, :]                     # [H, P] (C' = Cre + i Cim)
        for half in range(2):
            psl = slice(128 * half, 128 * (half + 1))
            bw[:, J, 0, half] = BJ.real[psl].T     # lhsT [h, p]
            bw[:, J, 1, half] = BJ.imag[psl].T
            # out = Re{C'_T x} = CTre*xr - CTim*xi ; rhs [p, h]
            cwt[:, J, 0, half] = CT.real[:, psl].T
            cwt[:, J, 1, half] = -CT.imag[:, psl].T

    rcol = np.zeros((128, 2), np.float32)
    rcol[:, 0] = r[:128]
    rcol[:, 1] = r[128:]
    dwm = np.diag(D.astype(np.float64)).astype(np.float32)
    eye = np.eye(128, dtype=np.float32)
    return dict(bw=bw, cw=cwt, epre=epre, epost=epost, rcol=rcol, dw=dwm,
                eye=eye)


def kernel(inputs, A_diag, G_diag, steps, B, C, D):
    from concourse import bass_utils

    inputs = np.asarray(inputs, np.float32)
    consts = _host_constants(np.asarray(A_diag), np.asarray(G_diag),
                             np.asarray(steps), np.asarray(B), np.asarray(C),
                             np.asarray(D))

    if "prog" not in _COMPILED:
        _COMPILED["prog"] = _build_program()
    nc = _COMPILED["prog"]

    in_maps = []
    for core in range(N_CORES):
        m = dict(consts)
        m["xin"] = np.ascontiguousarray(inputs[BPC * core: BPC * (core + 1)])
        in_maps.append(m)
    res = bass_utils.run_bass_kernel_spmd(nc, in_maps,
                                          core_ids=list(range(N_CORES)))
    out = np.concatenate([res.results[i]["out"] for i in range(N_CORES)],
                         axis=0)
    return out.astype(np.float32)


# revision 11
# speedup vs baseline: 9333.3284x; 1.0903x over previous
"""DampedLinOSSLayer Trainium2 kernel (8 NeuronCores, batch-sharded).

Math: per SSM channel p, the complex diagonal recurrence
    x_t = lam_p * x_{t-1} + bu_t,   lam_p = r_p * exp(i*th_p)
is factored through the gauge x_t = exp(i*th_p*t) * y_t:
    y_t = r_p * y_{t-1} + c_t,      c_t = exp(-i*th_p*t) * bu_t
which has a REAL per-channel coefficient -> runs as hardware
tensor_tensor_scan (DVE) on the re/im planes independently.
The phase rotations exp(-/+ i*th*t) are split as t = 512*T + t0:
the chunk part exp(+-i*th*512T) is folded (on host) into per-chunk
copies of the B / C projection weights; only the in-chunk part
exp(+-i*th*t0), t0 in [0,512), is applied on-device as elementwise
multiplies with constant [128, 512] tables.

Layout on device ("ST-form"): SSM channel p on partitions (2 halves of
128), time on the free dim. Per core: 4 batches of the 32.
  - input tiles [l,h] -> PE transpose -> inT [h, l]
  - B-proj:  bu[p_half, t]  = B_J^T.T @ inT        (PE)
  - pre-rotation (packed complex mul)              (DVE)
  - scan y = r*y + c along t, full L=2048          (DVE tensor_tensor_scan)
  - post-rotation -> x                             (DVE)
  - C-proj + D-residual -> out[t, h]               (PE, PSUM-accumulated)
"""

import functools
import numpy as np

BATCH, LENGTH, HIDDEN, P = 32, 2048, 128, 256
N_CORES = 8
BPC = BATCH // N_CORES          # batches per core
CH = 512                        # chunk size (phase fold granularity)
NCH = LENGTH // CH              # 4 chunks
NBLK = CH // 128                # 4 token-blocks of 128 per chunk

_COMPILED = {}


def _build_program(mm_dtype_name="float32", reps=1, skip=()):
    """reps>1 wraps the whole per-core body in a hardware loop (timing).
    skip: subset of {"dve", "pe_bc", "tr"} disabling sections (timing)."""
    import concourse.bacc as bacc
    import concourse.mybir as mybir
    from concourse.tile import TileContext

    f32 = mybir.dt.float32
    mmdt = getattr(mybir.dt, mm_dtype_name)

    nc = bacc.Bacc("TRN2", target_bir_lowering=False, debug=False,
                   num_devices=N_CORES)

    # ---- DRAM tensors (per-core) ----
    xin = nc.dram_tensor("xin", [BPC, LENGTH, HIDDEN], f32,
                         kind="ExternalInput").ap()
    # B weights, phase-folded per chunk: [J, comp(re/im), half, h, p]
    bw = nc.dram_tensor("bw", [HIDDEN, NCH, 2, 2, 128], f32,
                        kind="ExternalInput").ap()
    # C weights, phase-folded per chunk (sign of im folded): [T, comp, half, p, h]
    cw = nc.dram_tensor("cw", [128, NCH, 2, 2, HIDDEN], f32,
                        kind="ExternalInput").ap()
    # in-chunk rotation tables, packed for the 2-mult complex trick:
    # epre/epost: [half, which(0=T1,1=T2), 128, 2, 512]
    epre = nc.dram_tensor("epre", [128, 2, 2, 2, CH], f32,
                          kind="ExternalInput").ap()
    epost = nc.dram_tensor("epost", [128, 2, 2, 2, CH], f32,
                           kind="ExternalInput").ap()
    rcol = nc.dram_tensor("rcol", [128, 2], f32, kind="ExternalInput").ap()
    dw = nc.dram_tensor("dw", [HIDDEN, HIDDEN], f32, kind="ExternalInput").ap()
    eye = nc.dram_tensor("eye", [128, 128], f32, kind="ExternalInput").ap()
    out = nc.dram_tensor("out", [BPC, LENGTH, HIDDEN], f32,
                         kind="ExternalOutput").ap()

    with TileContext(nc) as tc:
        import contextlib

        @contextlib.contextmanager
        def body_loop():
            if reps == 1:
                yield
            else:
                with tc.For_i(0, reps, 1):
                    yield

        with (
            tc.tile_pool(name="const", bufs=1) as cpool,
            tc.tile_pool(name="inat", bufs=4) as inat_pool,
            tc.tile_pool(name="intp", bufs=2) as intr_pool,   # inT per batch
            tc.tile_pool(name="cbuf", bufs=2) as cbuf_pool,   # scan in
            tc.tile_pool(name="ybuf", bufs=2) as ybuf_pool,   # scan out
            tc.tile_pool(name="xbuf", bufs=2) as xbuf_pool,   # post-rot
            tc.tile_pool(name="pst", bufs=2, space="PSUM") as pst,
            tc.tile_pool(name="psb", bufs=1, space="PSUM") as psb,
            tc.tile_pool(name="pso", bufs=2, space="PSUM") as pso,
        ):
            # ---- constants to SBUF ----
            bw_t = cpool.tile([HIDDEN, NCH, 2, 2, 128], f32, tag="bw")
            cw_t = cpool.tile([128, NCH, 2, 2, HIDDEN], f32, tag="cw")
            epre_t = cpool.tile([128, 2, 2, 2, CH], f32, tag="epre")
            epost_t = cpool.tile([128, 2, 2, 2, CH], f32, tag="epost")
            rcol_t = cpool.tile([128, 2], f32, tag="rcol")
            dw_t = cpool.tile([HIDDEN, HIDDEN], f32, tag="dw")
            eye_t = cpool.tile([128, 128], f32, tag="eye")
            for src, dst in [(bw, bw_t), (cw, cw_t), (epre, epre_t),
                             (epost, epost_t), (rcol, rcol_t), (dw, dw_t),
                             (eye, eye_t)]:
                nc.sync.dma_start(dst[:], src[:])

            # broadcast r along free dim for the scan coefficient
            rbc = cpool.tile([128, 2, CH], f32, tag="rbc")
            for half in range(2):
                nc.vector.memset(rbc[:, half], 1.0)
                nc.vector.tensor_scalar_mul(
                    rbc[:, half], rbc[:, half], rcol_t[:, half:half + 1])

            ctx_loop = body_loop()
            ctx_loop.__enter__()
            for b in range(BPC):
                # ---- load + transpose input: inT [h, l] ----
                inT = intr_pool.tile([HIDDEN, LENGTH], f32, tag="inT")
                for blk in range(LENGTH // 128):
                    nat = inat_pool.tile([128, HIDDEN], f32, tag="nat")
                    nc.sync.dma_start(
                        nat[:], xin[b, 128 * blk:128 * (blk + 1), :])
                    if "tr" not in skip:
                        tp = pst.tile([HIDDEN, 128], f32, tag="tp")
                        nc.tensor.transpose(tp[:], nat[:], eye_t[:])
                        nc.scalar.copy(
                            inT[:, 128 * blk:128 * (blk + 1)], tp[:])

                y_prev = [None, None]
                for J in range(NCH):
                    tsl = slice(CH * J, CH * (J + 1))
                    x_sb = []
                    for half in range(2):
                        # ---- B-proj: bu[p_half, 2, CH] (re|im packed) ----
                        bu = psb.tile([128, 2, CH], f32, tag=f"bu{half}")
                        if "pe_bc" in skip:
                            nc.scalar.memzero(bu[:])
                        else:
                            for comp in range(2):
                                nc.tensor.matmul(
                                    bu[:, comp, :],
                                    bw_t[:, J, comp, half].bitcast(mmdt),
                                    inT[:, tsl].bitcast(mmdt),
                                    start=True, stop=True)
                        # ---- pre-rotation: c = E- * bu (complex) ----
                        if "dve" not in skip:
                            t1 = xbuf_pool.tile([128, 2, CH], f32, tag="t1")
                            t2 = xbuf_pool.tile([128, 2, CH], f32, tag="t2")
                            cc = cbuf_pool.tile(
                                [128, 2, CH], f32, tag=f"c{half}",
                                name=f"c{half}")
                            nc.vector.tensor_mul(
                                t1[:], bu[:], epre_t[:, half, 0])
                            nc.vector.tensor_mul(
                                t2[:], bu[:], epre_t[:, half, 1])
                            nc.vector.tensor_add(
                                cc[:, 0, :], t1[:, 0, :], t1[:, 1, :])
                            nc.vector.tensor_add(
                                cc[:, 1, :], t2[:, 0, :], t2[:, 1, :])

                        # ---- chained scan: y = r * y_prev + c ----
                        if "dve" not in skip:
                            yy = ybuf_pool.tile(
                                [128, 2, CH], f32, tag=f"y{half}",
                                name=f"y{half}")
                            for comp in range(2):
                                init = (0.0 if y_prev[half] is None else
                                        y_prev[half][:, comp, CH - 1:CH])
                                nc.vector.tensor_tensor_scan(
                                    yy[:, comp, :],
                                    rbc[:, half],
                                    cc[:, comp, :],
                                    init,
                                    op0=mybir.AluOpType.mult,
                                    op1=mybir.AluOpType.add)
                            y_prev[half] = yy

                        # ---- post-rotation: x = E+ * y (complex) ----
                        xs = xbuf_pool.tile([128, 2, CH], f32, tag=f"x{half}",
                                            name=f"x{half}")
                        if "dve" in skip:
                            nc.gpsimd.memset(xs[:], 0.0)
                        else:
                            t3 = xbuf_pool.tile([128, 2, CH], f32, tag="t3")
                            t4 = xbuf_pool.tile([128, 2, CH], f32, tag="t4")
                            nc.vector.tensor_mul(
                                t3[:], yy[:], epost_t[:, half, 0])
                            nc.vector.tensor_mul(
                                t4[:], yy[:], epost_t[:, half, 1])
                            nc.vector.tensor_add(
                                xs[:, 0, :], t3[:, 0, :], t3[:, 1, :])
                            nc.vector.tensor_add(
                                xs[:, 1, :], t4[:, 0, :], t4[:, 1, :])
                        x_sb.append(xs)

                    # ---- C-proj + D-residual per 128-token block ----
                    for i in range(NBLK):
                        bsl = slice(128 * i, 128 * (i + 1))
                        gsl = slice(CH * J + 128 * i, CH * J + 128 * (i + 1))
                        ops = pso.tile([128, HIDDEN], f32, tag="ops")
                        if "pe_bc" in skip:
                            nc.scalar.memzero(ops[:])
                        if "pe_bc" not in skip:
                            first = True
                            for comp in range(2):
                                for half in range(2):
                                    nc.tensor.matmul(
                                        ops[:],
                                        x_sb[half][:, comp, bsl].bitcast(mmdt),
                                        cw_t[:, J, comp, half].bitcast(mmdt),
                                        start=first, stop=False)
                                    first = False
                            nc.tensor.matmul(
                                ops[:], inT[:, gsl].bitcast(mmdt),
                                dw_t[:].bitcast(mmdt), start=False, stop=True)
                        osb = xbuf_pool.tile([128, HIDDEN], f32, tag="osb",
                                             bufs=4)
                        nc.scalar.copy(osb[:], ops[:])
                        nc.sync.dma_start(out[b, gsl, :], osb[:])

            ctx_loop.__exit__(None, None, None)

    nc.compile()
    return nc


def _host_constants(A_diag, G_diag, steps, B, C, D):
    """Parameter projection + eigenvalues + phase-folded weight tables."""
    A = A_diag.astype(np.float64)
    G = G_diag.astype(np.float64)
    st = steps.astype(np.float64)
    step = 1.0 / (1.0 + np.exp(-st))
    g = np.maximum(G, 0.0)
    denom = np.maximum(step * step, 1e-6)
    s = step * g
    base = np.sqrt(np.maximum(1.0 + s, 1e-6))
    a_low = (2.0 + s - 2.0 * base) / denom
    a_high = (2.0 + s + 2.0 * base) / denom
    a = a_low + np.maximum(A - a_low, 0.0) - np.maximum(A - a_high, 0.0)
    S = 1.0 / (1.0 + step * g)
    T = S + 1.0 - step * step * S * a
    imag = np.sqrt(np.maximum(S - 0.25 * T * T, 0.0))
    lam = 0.5 * T + 1j * imag                      # [P] complex128
    r = np.abs(lam)
    th = np.angle(lam)

    j0 = np.arange(CH, dtype=np.float64)
    # in-chunk rotations, [P, CH]
    cos_m = np.cos(th[:, None] * j0[None, :])
    sin_m = np.sin(th[:, None] * j0[None, :])

    # epre packs for c = exp(-i th t0') * bu:
    #   cre = bur*cos + bui*sin   -> T1 mult table [cos | sin], add halves
    #   cim = bur*(-sin) + bui*cos-> T2 mult table [-sin | cos], add halves
    # epost for x = exp(+i th t0') * y:
    #   xr = yr*cos + yi*(-sin)   -> T1 table [cos | -sin]
    #   xi = yr*sin + yi*cos      -> T2 table [sin | cos]
    epre = np.zeros((128, 2, 2, 2, CH), np.float32)
    epost = np.zeros((128, 2, 2, 2, CH), np.float32)
    for half in range(2):
        psl = slice(128 * half, 128 * (half + 1))
        epre[:, half, 0, 0, :] = cos_m[psl]
        epre[:, half, 0, 1, :] = sin_m[psl]
        epre[:, half, 1, 0, :] = -sin_m[psl]
        epre[:, half, 1, 1, :] = cos_m[psl]
        epost[:, half, 0, 0, :] = cos_m[psl]
        epost[:, half, 0, 1, :] = -sin_m[psl]
        epost[:, half, 1, 0, :] = sin_m[psl]
        epost[:, half, 1, 1, :] = cos_m[psl]

    # chunk-folded B: B_J = exp(-i th * CH * J) * (Br + i Bi)
    Bc = B[..., 0].astype(np.float64) + 1j * B[..., 1].astype(np.float64)
    Cc = C[..., 0].astype(np.float64) + 1j * C[..., 1].astype(np.float64)
    bw = np.zeros((HIDDEN, NCH, 2, 2, 128), np.float32)
    cwt = np.zeros((128, NCH, 2, 2, HIDDEN), np.float32)
    for J in range(NCH):
        ph = np.exp(-1j * th * (CH * J))           # [P]
        BJ = Bc * ph[:, None]                      # [P, H]
        phc = np.exp(+1j * th * (CH * J))
        CT = Cc * phc[None, :]                     # [H, P] (C' = Cre + i Cim)
        for half in range(2):
            psl = slice(128 * half, 128 * (half + 1))
            bw[:, J, 0, half] = BJ.real[psl].T     # lhsT [h, p]
            bw[:, J, 1, half] = BJ.imag[psl].T
            # out = Re{C'_T x} = CTre*xr - CTim*xi ; rhs [p, h]
            cwt[:, J, 0, half] = CT.real[:, psl].T
            cwt[:, J, 1, half] = -CT.imag[:, psl].T

    rcol = np.zeros((128, 2), np.float32)
    rcol[:, 0] = r[:128]
    rcol[:, 1] = r[128:]
    dwm = np.diag(D.astype(np.float64)).astype(np.float32)
    eye = np.eye(128, dtype=np.float32)
    return dict(bw=bw, cw=cwt, epre=epre, epost=epost, rcol=rcol, dw=dwm,
                eye=eye)


def kernel(inputs, A_diag, G_diag, steps, B, C, D):
    from concourse import bass_utils

    inputs = np.asarray(inputs, np.float32)
    consts = _host_constants(np.asarray(A_diag), np.asarray(G_diag),
                             np.asarray(steps), np.asarray(B), np.asarray(C),
                             np.asarray(D))

    if "prog" not in _COMPILED:
        _COMPILED["prog"] = _build_program()
    nc = _COMPILED["prog"]

    in_maps = []
    for core in range(N_CORES):
        m = dict(consts)
        m["xin"] = np.ascontiguousarray(inputs[BPC * core: BPC * (core + 1)])
        in_maps.append(m)
    res = bass_utils.run_bass_kernel_spmd(nc, in_maps,
                                          core_ids=list(range(N_CORES)))
    out = np.concatenate([res.results[i]["out"] for i in range(N_CORES)],
                         axis=0)
    return out.astype(np.float32)
